# revision 83
# baseline (speedup 1.0000x reference)
"""Trainium2 Bass kernel for nn_BlockedMLP (dense_mlp, 8 cores).

Strategy:
  - 8-way data parallel over the batch (B=2048 -> 256 rows/core), weights
    replicated. No collectives.
  - The BSR fc2 (50% block density, 32x32 blocks) is scattered into a dense
    [H, H] matrix on the host: on the PE array a matmul costs N streamed
    columns regardless of contraction K, so 32x32 sparse blocks waste ~4x
    throughput vs dense 128x128 tiles and the block gather costs more than
    the 2x FLOP saving.
  - Feature-major ("transposed") layout throughout: activations live in SBUF
    as [feature_partition, batch_free]; weights are the stationary matmul
    operand, activations stream. Host pre-transposes x and the weights.
  - ALL weights stream from HBM as INT8 with per-[tile,row] max-abs scales
    (host-quantized) and are dequantized on-device to bf16 by VectorE
    (tensor_scalar_mul, ~745ns per [128,1024] tile); matmuls run bf16 with
    fp32 PSUM. Per-core weight traffic drops 16.8MB -> 4.2MB: with all 8
    cores pulling ~2.3TB/s aggregate, the bf16 stream sat at the chip's
    HBM ceiling and a different core lost arbitration every run (stall ->
    HAM clock derate -> +5..9us max-core). int8 runs far below the ceiling
    and adds only ~0.9% to the 1.3% rel err (gate is 2e-2). fp8 weights
    (both-operand fp8 would halve PE time) fail the gate: 3.7-7% measured.
  - Every 8-output-tile wave runs as a 6-tile "A pass" (psum banks 0-5,
    642ns/k-tile demand, inline pair-DMA issue + dequant with a 4-tile
    lead) then a 2-tile "B pass" (banks 6-7) over the SAME retained bf16
    tiles. A pass's epilogues drain on ScalarE during the next pass's
    compute, so a new pass never write-after-read-waits an in-flight
    epilogue chain (an 8-at-once boundary stalled the PE 2-3.5us per wave
    trailing the epilogue cadence, often tripping the HAM derate).

    Scheduling rules found on HW (each worth 1-10us):
    - Dequant must be VectorE-only: an int8 input to the ACT engine
      hard-faults the exec unit (NRT_EXEC_UNIT_UNRECOVERABLE), and DVE
      epilogues ahead of dequants in DVE program order head-of-line-block
      the dequant stream at wave boundaries.
    - Epilogues (relu(ps+bias) / ps+bias) run on ScalarE-ACT: ~356ns
      pipelined cadence vs ~700ns on DVE; Relu+Identity share one ACT
      table set whose load rides a side DMA queue during fc1. ScalarE
      issues no weight DMAs (all int8 pairs ride the Sync HW queue), so
      nothing queues behind the epilogues. The final j7/j6 tail is split
      vector/scalar so the last tile's chain is the short scalar one.
    - The HAM clock governor grants full PE clock only after ~5us of
      gapless PE activity and derates after any multi-us idle: 26 dummy
      warmup matmuls bridge exactly until the first dequant (DMA-queue
      startup ~7.3us + 128KB transfer + ~1.3us completion-semaphore
      latency + 745ns dequant ~= 10.5-12.2us, core-dependent). At 20
      warmups the 0.8-1.2us handoff gap reset the governor on 1-2 cores
      per run (grant at +15.5 instead of +10, fc1 at half clock, +2.4us).
    - bias+scales (53KB) load FIRST on Sync: every dequant reads the
      scales; the gpsimd software-DGE load gated the first dequant ~2.6us
      late. x loads as ONE 512KB dma on Scalar. fc1's first four int8
      tiles go as single-tile DMAs so the PE restart gates on 128KB
      completions. fc3's 16 int8 tiles prefetch during fc2 wave 2 (one
      pair per odd k-tile) and dequantize just-in-time inside fc3's A
      pass; its B pass reuses the resident bf16 tiles.
    - Outputs store as bf16; wave-hidden stores ride gpsimd, tail-critical
      ones the two HW queues. The LightTail drops its trailing all-engine
      barrier too (-0.3us): NEFF completion already waits every engine's
      stream end, and Sync's drain carries the full semaphore wait list.
      Bass.reset()'s footer (barrier + NRT-expanded per-semaphore clears,
      ~1.4us in-window) must stay: the clears cannot race pending waits
      and the NEFF must be re-executable. The fixed NEFF preamble (~6.5us:
      3 barrier rounds + register TENSOR_LOADs) is immovable.

    Measured (8 cores, max-core NEFF exec): 72.88us max-core / 72.19us
    mean, spread 71.73-72.88 (session start: 84.1 max / 77.2 mean), rel
    err 1.33e-2 vs the 2e-2 gate (bf16 floor is 4.5e-3; int8 weights add
    ~0.9%). PE stream is gap-free on all cores: ~6.5us preamble + ~4us
    warmup/ramp + 54.8us of back-to-back matmuls + ~4.3us tail (epilogue,
    store, ~1.4us DMA-completion latency, reset footer).
"""

import numpy as np
import ml_dtypes

try:
    import concourse.bass as bass  # noqa: F401
except ImportError:
    import sys

    for _p in ("/opt/trn_rl_repo", "/root/.axon_site/_ro/trn_rl_repo"):
        if _p not in sys.path:
            sys.path.insert(0, _p)

import concourse.bacc as bacc
import concourse.bass as bass
import concourse.mybir as mybir
import concourse.tile as tile
from concourse import bass_utils

LIGHT_TAIL = True  # replace Tile's heavy end-of-kernel barrier with a minimal one
FAST_CONST = True  # route Bass-init const-AP memsets to VectorE (GpSimd is ~8x slower)

B, IN, H, OUT, BS = 2048, 1024, 2048, 1024, 32
NCORES = 8
BSH = B // NCORES  # 256 batch rows per core
P = 128
WCOLS = 1024  # streamed weight tile = [P, WCOLS] = 8 output tiles of 128

F32 = mybir.dt.float32
I8 = mybir.dt.int8
RELU = mybir.ActivationFunctionType.Relu
IDENT = mybir.ActivationFunctionType.Identity

# Wave schedule: (kt, n_out_tiles) per wave; weights packed in this order.
# ALL weights stream as int8 with per-[tile,row] scales, dequantized
# on-device to bf16: fc1 2 waves x 8 k-tiles, fc2 2 x 16, fc3 1 x 16.
NW1, NW2, NW3 = 2, 2, 1
KT1, KT2, KT3 = IN // P, H // P, H // P
WQ_TILES = NW1 * KT1 + NW2 * KT2 + NW3 * KT3  # 64 int8 tiles
Q_FC2, Q_FC3 = NW1 * KT1, NW1 * KT1 + NW2 * KT2  # stream offsets
NBIAS = 2 * H // P + OUT // P  # 40 bias columns
BCW = NBIAS + WQ_TILES  # bias + dequant scales

_CACHE = {}


def _emit(tc, DT, MMDT=None):
    """MMDT: optional matmul-operand dtype (e.g. float32r); operands are
    bitcast views, storage/DMA stay in DT."""
    nc = tc.nc
    mmcast = (lambda ap: ap.bitcast(MMDT)) if MMDT is not None else (lambda ap: ap)

    xT = nc.dram_tensor("xT", [P, KT1, BSH], DT, kind="ExternalInput").ap()
    wq = nc.dram_tensor("wq", [WQ_TILES, P, WCOLS], I8, kind="ExternalInput").ap()
    bc = nc.dram_tensor("bc", [P, BCW], F32, kind="ExternalInput").ap()
    # Store the final output in bf16 (host upcasts): halves the output DMA
    # bytes on the tail drain; the added rounding is ~0.2% vs the 2e-2 gate.
    ODT = DT if DT is mybir.dt.bfloat16 else F32
    outT = nc.dram_tensor("outT", [OUT // P, P, BSH], ODT, kind="ExternalOutput").ap()

    from contextlib import ExitStack

    with ExitStack() as ctx:
        wp = ctx.enter_context(tc.tile_pool(name="wpool", bufs=28))
        qp = ctx.enter_context(tc.tile_pool(name="qpool", bufs=8))
        q3p = ctx.enter_context(tc.tile_pool(name="q3pool", bufs=1))
        act = ctx.enter_context(tc.tile_pool(name="act", bufs=1))
        pp = ctx.enter_context(tc.tile_pool(name="ps", bufs=1, space="PSUM"))
        iop = ctx.enter_context(tc.tile_pool(name="io", bufs=1))

        # NOTE: the measured window START is pinned by the NEFF's own
        # preamble (~6.5us: engine barriers + register TENSOR_LOADs) —
        # data-DMA delay games cannot move it.
        warm_rhs = iop.tile([P, BSH], mybir.dt.bfloat16, tag="warm_rhs", name="warm_rhs")
        nc.vector.memset(warm_rhs[:], 0.0)
        # bias+scales load FIRST on the Sync HW queue (53KB, ~0.15us):
        # every dequant reads the scales, so the old gpsimd software-DGE
        # load (~2.6us slower) gated the first dequant and opened a
        # 1.2-3.9us PE gap at the warmup->real-work transition.
        bs = iop.tile([P, BCW], F32, tag="bs", name="bs")
        nc.sync.dma_start(bs[:], bc[:])
        xt = iop.tile([P, KT1, BSH], DT, tag="x", name="xt")
        # x in two halves: the first 256KB's completion lands ~0.7us
        # earlier, so xts[0..3] never co-gate the post-warmup restart on
        # a contended core. Scalar's queue has nothing behind these.
        nc.scalar.dma_start(xt[:, 0 : KT1 // 2, :], xT[:, 0 : KT1 // 2, :])
        nc.scalar.dma_start(xt[:, KT1 // 2 :, :], xT[:, KT1 // 2 :, :])
        xts = [xt[:, k, :] for k in range(KT1)]
        b1s = bs[:, 0 : H // P]
        b2s = bs[:, H // P : 2 * H // P]
        b3s = bs[:, 2 * H // P : NBIAS]
        scs = bs[:, NBIAS:]  # per-[tile,row] int8 dequant scales, wq order

        # PE warmup: the HAM clock governor grants full speed only after
        # ~5us of UNINTERRUPTED PE activity — any sub-us stall resets the
        # counter. Real matmuls during the ramp inevitably micro-stall on
        # the trickling weight stream, so the grant slips and everything
        # before it runs at half clock. 22 dependency-free warmups give a
        # contiguous activity block (grant ~+11.7) while the stream builds
        # a ~1.5MB lead; real work then starts at full clock and never
        # looks back.
        # Warmup accumulates in ps7's bank: fc1 wave 1's (k0, j7) is the
        # 8th matmul, so the WAR on the last warm matmul hides behind
        # j0..j6 instead of stalling the very first real matmul (~0.9us).
        warm_ps = pp.tile([P, BSH], F32, tag="ps7", name="warm_ps")
        for i in range(26):
            nc.tensor.matmul(
                warm_ps[:],
                mmcast(warm_rhs[:, 0:P]),
                mmcast(warm_rhs[:]),
                start=True,
                stop=True,
            )

        def deqop(dst, src, col):
            """int8 -> bf16 dequant with per-[tile,row] scale, on VectorE
            only (~745ns measured per [P,WCOLS] tile, under the A pass's
            642ns/tile consumption once the 4-tile lead absorbs the
            difference; B passes refill the lead). DVE carries nothing
            else mid-kernel, so its deq stream free-runs ahead of the PE,
            throttled only by the int8 pool's rotation. (ACT cannot
            dequant: an int8 input to the activation unit hard-faults the
            exec unit.)
            """
            nc.vector.tensor_scalar_mul(dst, src, scs[:, col : col + 1])

        def epilogue(ps_tile, bias, bias_off, j, func, out_dt, tag, eng="scalar"):
            """func(ps + bias) on ScalarE-ACT (~356ns pipelined cadence; the
            DVE equivalent costs ~700ns and would also head-of-line-block
            the dequant stream). Scalar issues no weight DMAs after fc1
            (int8 rides Sync), so nothing queues behind the epilogues; the
            one-time ACT_TABLE_LOAD (Relu+Identity share a table set)
            hides under fc1 compute. eng="vector" only for the very last
            output tile, so the final two epilogues run in parallel.
            """
            o = act.tile([P, BSH], out_dt, tag=f"{tag}o{j}", name=f"{tag}o{j}")
            bias_ap = bias[:, bias_off + j : bias_off + j + 1]
            if eng == "scalar":
                nc.scalar.activation(
                    o[:], ps_tile[:], RELU if func is RELU else IDENT, bias=bias_ap
                )
            elif func is RELU:
                nc.vector.tensor_scalar(
                    o[:],
                    ps_tile[:],
                    bias_ap,
                    0.0,
                    mybir.AluOpType.add,
                    mybir.AluOpType.max,
                )
            else:
                nc.vector.tensor_scalar_add(o[:], ps_tile[:], bias_ap)
            return o[:]

        JA = (0, 1, 2, 3, 4, 5)  # 6-tile A pass: psum banks 0-5
        JB = (6, 7)  # 2-tile B pass: banks 6-7

        class QWave:
            """int8 wave. DMA issues, dequants, matmuls, and epilogues are
            separately sequenced so the caller controls each engine's
            program interleaving: pair issues are emitted INLINE with the
            matmul loop (issue rate ~750ns/pair vs PE 1712ns/pair, so the
            queue builds lead, bounded by the qpool rotation), and the
            NEXT wave's first issues+dequants are emitted before THIS
            wave's odd-j (VectorE) epilogues, keeping DVE's dequant lead
            alive across wave boundaries.
            """

            def __init__(self, bias_off, tag, qbase, kt=KT2):
                self.bias_off, self.tag, self.qbase, self.kt = bias_off, tag, qbase, kt
                self.ps = [
                    pp.tile([P, BSH], F32, tag=f"ps{i}", name=f"{tag}ps{i}")
                    for i in range(WCOLS // P)
                ]
                self.wqts = []
                self.wt = {}
                self.ndeq = 0

            def issue_pairs(self, n):
                # int8 pairs ride Sync ONLY: Scalar's post-fc1 stream is
                # epilogues, which would head-of-line-block DMAs behind.
                while len(self.wqts) < min(n, self.kt // 2):
                    p = len(self.wqts)
                    wqt = qp.tile([P, 2, WCOLS], I8, tag="wq", name=f"{self.tag}q{p}")
                    src = wq[self.qbase + 2 * p : self.qbase + 2 * p + 2].rearrange(
                        "i p c -> p i c"
                    )
                    if self.qbase == 0 and p < 2:
                        # The kernel's first four weight tiles go as
                        # single-tile DMAs: each 128KB completion lands
                        # separately, so dequants 0-3 — the gates for the
                        # first post-warmup matmuls — pace the PE's restart
                        # even when the startup HBM burst is contended.
                        nc.sync.dma_start(wqt[:, 0, :], src[:, 0, :])
                        nc.sync.dma_start(wqt[:, 1, :], src[:, 1, :])
                    else:
                        nc.sync.dma_start(wqt[:], src)
                    self.wqts.append(wqt)

            def deq_upto(self, n):
                while self.ndeq < min(n, self.kt):
                    k = self.ndeq
                    self.issue_pairs(k // 2 + 1)
                    w = wp.tile([P, 1, WCOLS], DT, tag="w", name=f"{self.tag}w{k}")
                    deqop(w[:, 0, :], self.wqts[k // 2][:, k % 2, :], self.qbase + k)
                    self.wt[k] = (w, 0)
                    self.ndeq += 1

            def mms_a(self, rhs_tiles, lead=6, plead=4, hook=None):
                """A pass (j0-5, 642ns/k-tile) with inline issue+dequant;
                dequanted tiles are RETAINED for the B pass."""
                for k in range(self.kt):
                    self.issue_pairs(k // 2 + plead)
                    self.deq_upto(k + lead)
                    if hook is not None:
                        hook(k)
                    w, kk = self.wt[k]
                    for j in JA:
                        nc.tensor.matmul(
                            self.ps[j][:],
                            mmcast(w[:, kk, j * P : (j + 1) * P]),
                            mmcast(rhs_tiles[k]),
                            start=(k == 0),
                            stop=(k == self.kt - 1),
                        )

            def mms_b(self, rhs_tiles):
                """B pass (j6-7) on the retained tiles: no dequant demand,
                so DVE builds lead for the next wave."""
                for k in range(self.kt):
                    w, kk = self.wt[k]
                    for j in JB:
                        nc.tensor.matmul(
                            self.ps[j][:],
                            mmcast(w[:, kk, j * P : (j + 1) * P]),
                            mmcast(rhs_tiles[k]),
                            start=(k == 0),
                            stop=(k == self.kt - 1),
                        )

            def epis(self, js, bias, out_dt):
                return [
                    epilogue(self.ps[j], bias, self.bias_off, j, RELU, out_dt, self.tag)
                    for j in js
                ]

        # Every wave runs as a 6-tile A pass then a 2-tile B pass over the
        # same retained weight tiles. A pass's epilogues drain on Scalar
        # during the following pass's compute, so no pass ever WARs an
        # in-flight epilogue chain — the 8-at-once wave boundary used to
        # stall the PE 2-3.5us (and trip HAM derates) while its restart
        # trailed the epilogue cadence.
        w1a = QWave(0, "l1w0", 0, kt=KT1)
        w1a.deq_upto(4)
        w1a.mms_a(xts)
        w1a.mms_b(xts)
        w1b = QWave(8, "l1w1", KT1, kt=KT1)
        w1b.deq_upto(4)
        hts = w1a.epis(JA, b1s, DT) + w1a.epis(JB, b1s, DT)
        w1b.mms_a(xts)
        w1b.mms_b(xts)
        qw1 = QWave(0, "l2w0", Q_FC2)
        qw1.deq_upto(6)
        hts += w1b.epis(JA, b1s, DT) + w1b.epis(JB, b1s, DT)
        qw1.mms_a(hts)
        qw1.mms_b(hts)

        qw2 = QWave(8, "l2w1", Q_FC2 + KT2)
        qw2.deq_upto(6)
        h2s = qw1.epis(JA, b2s, DT) + qw1.epis(JB, b2s, DT)

        # fc3 int8 prefetch interleaved into fc2 wave 2's issue stream on
        # Sync (one q3 pair per odd k): data lands during fc2 compute
        # without delaying fc2's own pairs.
        q3tiles = []

        def q3_hook(k):
            if k % 2 == 1 and len(q3tiles) < KT3 // 2:
                p = len(q3tiles)
                t3 = q3p.tile([P, 2, WCOLS], I8, tag=f"q3_{p}", name=f"q3_{p}", bufs=1)
                src = wq[Q_FC3 + 2 * p : Q_FC3 + 2 * p + 2].rearrange("i p c -> p i c")
                nc.sync.dma_start(t3[:], src)
                q3tiles.append(t3)

        qw2.mms_a(hts, hook=q3_hook)
        qw2.mms_b(hts)

        w3bf = {}

        def deq3(k):
            w = wp.tile([P, 1, WCOLS], DT, tag=f"w3_{k}", name=f"w3_{k}", bufs=1)
            deqop(w[:, 0, :], q3tiles[k // 2][:, k % 2, :], Q_FC3 + k)
            w3bf[k] = (w, 0)

        # fc3 wave A's first dequants run on DVE right after fc2 wave 2's
        # (during fc2's B pass), while the PE is still in fc2.
        for k in range(4):
            deq3(k)
        h2s += qw2.epis(JA, b2s, DT) + qw2.epis(JB, b2s, DT)

        # fc3: wave A = j0-5 k-outer (6 matmuls = 642ns per k-tile vs the
        # ~700ns dequant, absorbed by the 4-tile lead); wave B = j6-7
        # k-inner reusing the now-resident bf16 tiles. Wave A's epilogues
        # + gpsimd stores hide under wave B's 3.4us of matmuls; the tail
        # is 2 parallel epilogues + 2 parallel HW-queue stores.
        psA = [pp.tile([P, BSH], F32, tag=f"ps{j}", name=f"l3ps{j}") for j in JA]
        for k in range(KT3):
            if k + 4 < KT3:
                deq3(k + 4)
            w, kk = w3bf[k]
            for jj, j in enumerate(JA):
                nc.tensor.matmul(
                    psA[jj][:],
                    mmcast(w[:, kk, j * P : (j + 1) * P]),
                    mmcast(h2s[k]),
                    start=(k == 0),
                    stop=(k == KT3 - 1),
                )
        for jj, j in enumerate(JA):
            o = epilogue(psA[jj], b3s, 0, j, None, ODT, "l3")
            # j4/j5 store via the (idle) HW queues: the gpsimd software
            # queue's ~1.3us completion latency on the last A stores was
            # poking past wave B into the tail's completion wait.
            if j < 4:
                nc.gpsimd.dma_start(outT[j], o)
            else:
                (nc.sync if j == 4 else nc.scalar).dma_start(outT[j], o)

        # fc3's B pass runs j7's full k-loop FIRST, then j6's: j7's
        # (slower, VectorE) epilogue and its Sync store hide under j6's
        # 1.7us of matmuls, so the exec-critical tail is only j6's scalar
        # epilogue + Scalar-queue store + DMA completion.
        psB = {j: pp.tile([P, BSH], F32, tag=f"ps{j}", name=f"l3ps{j}") for j in JB}
        for j, eng, store_q in ((7, "vector", nc.sync), (6, "scalar", nc.scalar)):
            for k in range(KT3):
                w, kk = w3bf[k]
                nc.tensor.matmul(
                    psB[j][:],
                    mmcast(w[:, kk, j * P : (j + 1) * P]),
                    mmcast(h2s[k]),
                    start=(k == 0),
                    stop=(k == KT3 - 1),
                )
            o = epilogue(psB[j], b3s, 0, j, None, ODT, "l3", eng)
            store_q.dma_start(outT[j], o)


class _LightTailTileContext(tile.TileContext):
    """TileContext with a minimal end-of-kernel sequence.

    Tile's default tail (drain + full all-engine barrier + DMA/semaphore
    reset + second barrier) costs ~8-10us on HW, dominated by NRT's
    expansion of the drain-with-sem-range reset. For a single-TileContext
    kernel the correctness requirement at the end is just: all engines done
    and all output DMAs complete before the NEFF signals completion.
    """

    def _drain_and_barrier(self, tick_clock, wait_clock):
        if not hasattr(self.nc, "_tile_sem_poison_stack"):
            return super()._drain_and_barrier(tick_clock, wait_clock)
        from concourse.vector_clock import ScopedClock

        drain_inst = self.nc.sync.drain()
        wait_clock.add_sem_waits(
            drain_inst.ins, ScopedClock({None: tick_clock.global_clock})
        )
        # No trailing all-engine barrier: NEFF completion already waits
        # for every engine's stream end, and Sync's drain (with the full
        # semaphore wait list above) covers all tracked DMA completions.
        # The barrier's two $S[2] rendezvous rounds cost ~0.7us in-window.
        assert self.sems is not None
        popped = self.nc._tile_sem_poison_stack.pop()
        assert popped is self._sem_poison


def _build(dt_name):
    if dt_name in _CACHE:
        return _CACHE[dt_name]
    DT = {"bf16": mybir.dt.bfloat16, "f32r": mybir.dt.float32r, "f32": F32}[dt_name]
    MMDT = None

    patches = []
    if FAST_CONST:
        try:
            import concourse.bass as cbass

            # During Bass construction only, reroute GpSimd memsets (the
            # framework's const-AP init) to the much faster VectorE: they
            # gate the initial all-engine barrier.
            gps_cls = cbass.BassGpSimd

            def memset_shim(self, ap, constant):
                return self.bass.vector.memset(ap, constant)

            had = "memset" in vars(gps_cls)
            orig = vars(gps_cls).get("memset")
            gps_cls.memset = memset_shim
            patches.append((gps_cls, "memset", had, orig))
            # The barrier after const-AP init protects readers of the const
            # tiles; this kernel never reads them, so skip it.
            bar_orig = cbass.Bass.all_engine_barrier

            def bar_shim(self, *, sem_only=False):
                return None

            cbass.Bass.all_engine_barrier = bar_shim
            patches.append((cbass.Bass, "all_engine_barrier", True, bar_orig))
        except AttributeError:
            pass

    try:
        nc = bacc.Bacc(
            "TRN2",
            target_bir_lowering=False,
            debug=False,
            enable_asserts=False,
            num_devices=NCORES,
        )
    finally:
        for klass, attr, had, orig in patches:
            if had:
                setattr(klass, attr, orig)
            else:
                delattr(klass, attr)

    tc_cls = _LightTailTileContext if LIGHT_TAIL else tile.TileContext
    with tc_cls(nc) as tc:
        _emit(tc, DT, MMDT)
    nc.compile()
    _CACHE[dt_name] = nc
    return nc


def _np_dt(dt_name):
    return mybir.dt.np({"bf16": mybir.dt.bfloat16, "f32r": F32, "f32": F32}[dt_name])


def _host_prep(x, W1, b1, crow_indices, col_indices, values, b2, W3, b3, npdt):
    rb = crow_indices.shape[0] - 1
    nnz, bs, _ = values.shape
    cb = H // bs
    # Scatter BSR into dense W2 [H, H].
    blocks = np.zeros((rb, cb, bs, bs), np.float32)
    row_ids = (
        np.searchsorted(crow_indices, np.arange(nnz, dtype=np.int64), side="right") - 1
    )
    blocks[row_ids, col_indices] = values
    W2 = blocks.transpose(0, 2, 1, 3).reshape(H, H)

    # Pack the streamed weight sequences: for each layer, for each wave
    # (column-half), the k-tiles [P, WCOLS] in consumption order.
    def waves(wT, kdim, nw, dt):  # wT [kdim, ndim] -> [nw*kt, P, WCOLS]
        kt = kdim // P
        t = wT.reshape(kt, P, nw, WCOLS).astype(dt)
        return np.ascontiguousarray(t.transpose(2, 0, 1, 3).reshape(nw * kt, P, WCOLS))

    # ALL weights stream as int8 with per-[tile,row] max-abs scales: the
    # contended HBM weight bytes drop 4x; dequant to bf16 is on-device.
    wq_f = np.concatenate(
        [
            waves(np.ascontiguousarray(W1.T), IN, NW1, np.float32),
            waves(np.ascontiguousarray(W2.T), H, NW2, np.float32),
            waves(np.ascontiguousarray(W3.T), H, NW3, np.float32),
        ]
    )  # [64, P, WCOLS] fp32
    sc = np.abs(wq_f).max(axis=2) / 127.0  # [64, P]
    sc = np.maximum(sc, 1e-30)
    wq8 = np.clip(np.rint(wq_f / sc[:, :, None]), -127, 127).astype(np.int8)
    bc = np.ascontiguousarray(
        np.concatenate(
            [
                b1.reshape(H // P, P).T.astype(np.float32),
                b2.reshape(H // P, P).T.astype(np.float32),
                b3.reshape(OUT // P, P).T.astype(np.float32),
                sc.T.astype(np.float32),
            ],
            axis=1,
        )
    )
    # x -> per-core transposed shards, [P, kt, BSH], natural k order.
    xT_all = np.ascontiguousarray(x.T.astype(npdt))  # [IN, B]
    shards = [
        np.ascontiguousarray(
            xT_all[:, c * BSH : (c + 1) * BSH].reshape(KT1, P, BSH).transpose(1, 0, 2)
        )
        for c in range(NCORES)
    ]
    shared = dict(wq=wq8, bc=bc)
    return [dict(shared, xT=shards[c]) for c in range(NCORES)]


def kernel(x, W1, b1, crow_indices, col_indices, values, b2, W3, b3, _dt="bf16"):
    nc = _build(_dt)
    in_maps = _host_prep(
        np.asarray(x, np.float32),
        np.asarray(W1, np.float32),
        np.asarray(b1, np.float32),
        np.asarray(crow_indices),
        np.asarray(col_indices),
        np.asarray(values, np.float32),
        np.asarray(b2, np.float32),
        np.asarray(W3, np.float32),
        np.asarray(b3, np.float32),
        _np_dt(_dt),
    )
    res = bass_utils.run_bass_kernel_spmd(nc, in_maps, core_ids=list(range(NCORES)))
    out = np.concatenate(
        [res.results[c]["outT"].reshape(OUT, BSH).T for c in range(NCORES)], axis=0
    )
    return np.ascontiguousarray(out.astype(np.float32))



# revision 85
# speedup vs baseline: 1.0189x; 1.0189x over previous
"""Trainium2 Bass kernel for nn_BlockedMLP (dense_mlp, 8 cores).

Strategy:
  - 8-way data parallel over the batch (B=2048 -> 256 rows/core), weights
    replicated. No collectives.
  - The BSR fc2 (50% block density, 32x32 blocks) is scattered into a dense
    [H, H] matrix on the host: on the PE array a matmul costs N streamed
    columns regardless of contraction K, so 32x32 sparse blocks waste ~4x
    throughput vs dense 128x128 tiles and the block gather costs more than
    the 2x FLOP saving.
  - Feature-major ("transposed") layout throughout: activations live in SBUF
    as [feature_partition, batch_free]; weights are the stationary matmul
    operand, activations stream. Host pre-transposes x and the weights.
  - ALL weights stream from HBM as INT8 with per-[tile,row] max-abs scales
    (host-quantized) and are dequantized on-device to bf16 by VectorE
    (tensor_scalar_mul, ~745ns per [128,1024] tile); matmuls run bf16 with
    fp32 PSUM. Per-core weight traffic drops 16.8MB -> 4.2MB: with all 8
    cores pulling ~2.3TB/s aggregate, the bf16 stream sat at the chip's
    HBM ceiling and a different core lost arbitration every run (stall ->
    HAM clock derate -> +5..9us max-core). int8 runs far below the ceiling
    and adds only ~0.9% to the 1.3% rel err (gate is 2e-2). fp8 weights
    (both-operand fp8 would halve PE time) fail the gate: 3.7-7% measured.
  - Every 8-output-tile wave runs as a 6-tile "A pass" (psum banks 0-5,
    642ns/k-tile demand, inline pair-DMA issue + dequant with a 4-tile
    lead) then a 2-tile "B pass" (banks 6-7) over the SAME retained bf16
    tiles. A pass's epilogues drain on ScalarE during the next pass's
    compute, so a new pass never write-after-read-waits an in-flight
    epilogue chain (an 8-at-once boundary stalled the PE 2-3.5us per wave
    trailing the epilogue cadence, often tripping the HAM derate).

    Scheduling rules found on HW (each worth 1-10us):
    - Dequant must be VectorE-only: an int8 input to the ACT engine
      hard-faults the exec unit (NRT_EXEC_UNIT_UNRECOVERABLE), and DVE
      epilogues ahead of dequants in DVE program order head-of-line-block
      the dequant stream at wave boundaries.
    - Epilogues (relu(ps+bias) / ps+bias) run on ScalarE-ACT: ~356ns
      pipelined cadence vs ~700ns on DVE; Relu+Identity share one ACT
      table set whose load rides a side DMA queue during fc1. ScalarE
      issues no weight DMAs (all int8 pairs ride the Sync HW queue), so
      nothing queues behind the epilogues. The final j7/j6 tail is split
      vector/scalar so the last tile's chain is the short scalar one.
    - The HAM clock governor grants full PE clock only after ~5us of
      gapless PE activity and derates after any multi-us idle: 26 dummy
      warmup matmuls bridge exactly until the first dequant (DMA-queue
      startup ~7.3us + 128KB transfer + ~1.3us completion-semaphore
      latency + 745ns dequant ~= 10.5-12.2us, core-dependent). At 20
      warmups the 0.8-1.2us handoff gap reset the governor on 1-2 cores
      per run (grant at +15.5 instead of +10, fc1 at half clock, +2.4us).
    - bias+scales (53KB) load FIRST on Sync: every dequant reads the
      scales; the gpsimd software-DGE load gated the first dequant ~2.6us
      late. x loads as ONE 512KB dma on Scalar. fc1's first four int8
      tiles go as single-tile DMAs so the PE restart gates on 128KB
      completions. fc3's 16 int8 tiles prefetch during fc2 wave 2 (one
      pair per odd k-tile) and dequantize just-in-time inside fc3's A
      pass; its B pass reuses the resident bf16 tiles.
    - Outputs store as bf16; wave-hidden stores ride gpsimd, tail-critical
      ones the two HW queues. The LightTail drops its trailing all-engine
      barrier too (-0.3us): NEFF completion already waits every engine's
      stream end, and Sync's drain carries the full semaphore wait list.
      Bass.reset()'s footer (barrier + NRT-expanded per-semaphore clears,
      ~1.4us in-window) must stay: the clears cannot race pending waits
      and the NEFF must be re-executable. The fixed NEFF preamble (~6.5us:
      3 barrier rounds + register TENSOR_LOADs) is immovable.

    Measured (8 cores, max-core NEFF exec): 72.88us max-core / 72.19us
    mean, spread 71.73-72.88 (session start: 84.1 max / 77.2 mean), rel
    err 1.33e-2 vs the 2e-2 gate (bf16 floor is 4.5e-3; int8 weights add
    ~0.9%). PE stream is gap-free on all cores: ~6.5us preamble + ~4us
    warmup/ramp + 54.8us of back-to-back matmuls + ~4.3us tail (epilogue,
    store, ~1.4us DMA-completion latency, reset footer).
"""

import numpy as np
import ml_dtypes

try:
    import concourse.bass as bass  # noqa: F401
except ImportError:
    import sys

    for _p in ("/opt/trn_rl_repo", "/root/.axon_site/_ro/trn_rl_repo"):
        if _p not in sys.path:
            sys.path.insert(0, _p)

import concourse.bacc as bacc
import concourse.bass as bass
import concourse.mybir as mybir
import concourse.tile as tile
from concourse import bass_utils

LIGHT_TAIL = True  # replace Tile's heavy end-of-kernel barrier with a minimal one
FAST_CONST = True  # route Bass-init const-AP memsets to VectorE (GpSimd is ~8x slower)

B, IN, H, OUT, BS = 2048, 1024, 2048, 1024, 32
NCORES = 8
BSH = B // NCORES  # 256 batch rows per core
P = 128
WCOLS = 1024  # streamed weight tile = [P, WCOLS] = 8 output tiles of 128

F32 = mybir.dt.float32
I8 = mybir.dt.int8
RELU = mybir.ActivationFunctionType.Relu
IDENT = mybir.ActivationFunctionType.Identity

# Wave schedule: (kt, n_out_tiles) per wave; weights packed in this order.
# ALL weights stream as int8 with per-[tile,row] scales, dequantized
# on-device to bf16: fc1 2 waves x 8 k-tiles, fc2 2 x 16, fc3 1 x 16.
NW1, NW2, NW3 = 2, 2, 1
KT1, KT2, KT3 = IN // P, H // P, H // P
WQ_TILES = NW1 * KT1 + NW2 * KT2 + NW3 * KT3  # 64 int8 tiles
Q_FC2, Q_FC3 = NW1 * KT1, NW1 * KT1 + NW2 * KT2  # stream offsets
NBIAS = 2 * H // P + OUT // P  # 40 bias columns
BCW = NBIAS + WQ_TILES  # bias + dequant scales

_CACHE = {}


def _emit(tc, DT, MMDT=None):
    """MMDT: optional matmul-operand dtype (e.g. float32r); operands are
    bitcast views, storage/DMA stay in DT."""
    nc = tc.nc
    mmcast = (lambda ap: ap.bitcast(MMDT)) if MMDT is not None else (lambda ap: ap)

    xT = nc.dram_tensor("xT", [P, KT1, BSH], DT, kind="ExternalInput").ap()
    wq = nc.dram_tensor("wq", [WQ_TILES, P, WCOLS], I8, kind="ExternalInput").ap()
    bc = nc.dram_tensor("bc", [P, BCW], F32, kind="ExternalInput").ap()
    # Store the final output in bf16 (host upcasts): halves the output DMA
    # bytes on the tail drain; the added rounding is ~0.2% vs the 2e-2 gate.
    ODT = DT if DT is mybir.dt.bfloat16 else F32
    outT = nc.dram_tensor("outT", [OUT // P, P, BSH], ODT, kind="ExternalOutput").ap()

    from contextlib import ExitStack

    with ExitStack() as ctx:
        wp = ctx.enter_context(tc.tile_pool(name="wpool", bufs=28))
        qp = ctx.enter_context(tc.tile_pool(name="qpool", bufs=8))
        q3p = ctx.enter_context(tc.tile_pool(name="q3pool", bufs=1))
        act = ctx.enter_context(tc.tile_pool(name="act", bufs=1))
        pp = ctx.enter_context(tc.tile_pool(name="ps", bufs=1, space="PSUM"))
        iop = ctx.enter_context(tc.tile_pool(name="io", bufs=1))

        # NOTE: the measured window START is pinned by the NEFF's own
        # preamble (~6.5us: engine barriers + register TENSOR_LOADs) —
        # data-DMA delay games cannot move it.
        warm_rhs = iop.tile([P, BSH], mybir.dt.bfloat16, tag="warm_rhs", name="warm_rhs")
        nc.vector.memset(warm_rhs[:], 0.0)
        # bias+scales load FIRST on the Sync HW queue (53KB, ~0.15us):
        # every dequant reads the scales, so the old gpsimd software-DGE
        # load (~2.6us slower) gated the first dequant and opened a
        # 1.2-3.9us PE gap at the warmup->real-work transition.
        bs = iop.tile([P, BCW], F32, tag="bs", name="bs")
        nc.sync.dma_start(bs[:], bc[:])
        xt = iop.tile([P, KT1, BSH], DT, tag="x", name="xt")
        # x in two halves: the first 256KB's completion lands ~0.7us
        # earlier, so xts[0..3] never co-gate the post-warmup restart on
        # a contended core. Scalar's queue has nothing behind these.
        nc.scalar.dma_start(xt[:, 0 : KT1 // 2, :], xT[:, 0 : KT1 // 2, :])
        nc.scalar.dma_start(xt[:, KT1 // 2 :, :], xT[:, KT1 // 2 :, :])
        xts = [xt[:, k, :] for k in range(KT1)]
        b1s = bs[:, 0 : H // P]
        b2s = bs[:, H // P : 2 * H // P]
        b3s = bs[:, 2 * H // P : NBIAS]
        scs = bs[:, NBIAS:]  # per-[tile,row] int8 dequant scales, wq order

        # PE warmup: the HAM clock governor grants full speed only after
        # ~5us of UNINTERRUPTED PE activity — any sub-us stall resets the
        # counter. Real matmuls during the ramp inevitably micro-stall on
        # the trickling weight stream, so the grant slips and everything
        # before it runs at half clock. 22 dependency-free warmups give a
        # contiguous activity block (grant ~+11.7) while the stream builds
        # a ~1.5MB lead; real work then starts at full clock and never
        # looks back.
        # Warmup accumulates in ps7's bank: fc1 wave 1's (k0, j7) is the
        # 8th matmul, so the WAR on the last warm matmul hides behind
        # j0..j6 instead of stalling the very first real matmul (~0.9us).
        warm_ps = pp.tile([P, BSH], F32, tag="ps7", name="warm_ps")
        for i in range(26):
            nc.tensor.matmul(
                warm_ps[:],
                mmcast(warm_rhs[:, 0:P]),
                mmcast(warm_rhs[:]),
                start=True,
                stop=True,
            )

        def deqop(dst, src, col):
            """int8 -> bf16 dequant with per-[tile,row] scale, on VectorE
            only (~745ns measured per [P,WCOLS] tile, under the A pass's
            642ns/tile consumption once the 4-tile lead absorbs the
            difference; B passes refill the lead). DVE carries nothing
            else mid-kernel, so its deq stream free-runs ahead of the PE,
            throttled only by the int8 pool's rotation. (ACT cannot
            dequant: an int8 input to the activation unit hard-faults the
            exec unit.)
            """
            nc.vector.tensor_scalar_mul(dst, src, scs[:, col : col + 1])

        def epilogue(ps_tile, bias, bias_off, j, func, out_dt, tag, eng="scalar"):
            """func(ps + bias) on ScalarE-ACT (~356ns pipelined cadence; the
            DVE equivalent costs ~700ns and would also head-of-line-block
            the dequant stream). Scalar issues no weight DMAs after fc1
            (int8 rides Sync), so nothing queues behind the epilogues; the
            one-time ACT_TABLE_LOAD (Relu+Identity share a table set)
            hides under fc1 compute. eng="vector" only for the very last
            output tile, so the final two epilogues run in parallel.
            """
            o = act.tile([P, BSH], out_dt, tag=f"{tag}o{j}", name=f"{tag}o{j}")
            bias_ap = bias[:, bias_off + j : bias_off + j + 1]
            if eng == "scalar":
                nc.scalar.activation(
                    o[:], ps_tile[:], RELU if func is RELU else IDENT, bias=bias_ap
                )
            elif func is RELU:
                nc.vector.tensor_scalar(
                    o[:],
                    ps_tile[:],
                    bias_ap,
                    0.0,
                    mybir.AluOpType.add,
                    mybir.AluOpType.max,
                )
            else:
                nc.vector.tensor_scalar_add(o[:], ps_tile[:], bias_ap)
            return o[:]

        JA = (0, 1, 2, 3, 4, 5)  # 6-tile A pass: psum banks 0-5
        JB = (6, 7)  # 2-tile B pass: banks 6-7

        class QWave:
            """int8 wave. DMA issues, dequants, matmuls, and epilogues are
            separately sequenced so the caller controls each engine's
            program interleaving: pair issues are emitted INLINE with the
            matmul loop (issue rate ~750ns/pair vs PE 1712ns/pair, so the
            queue builds lead, bounded by the qpool rotation), and the
            NEXT wave's first issues+dequants are emitted before THIS
            wave's odd-j (VectorE) epilogues, keeping DVE's dequant lead
            alive across wave boundaries.
            """

            def __init__(self, bias_off, tag, qbase, kt=KT2):
                self.bias_off, self.tag, self.qbase, self.kt = bias_off, tag, qbase, kt
                self.ps = [
                    pp.tile([P, BSH], F32, tag=f"ps{i}", name=f"{tag}ps{i}")
                    for i in range(WCOLS // P)
                ]
                self.wqts = []
                self.wt = {}
                self.ndeq = 0

            def issue_pairs(self, n):
                # int8 pairs ride Sync ONLY: Scalar's post-fc1 stream is
                # epilogues, which would head-of-line-block DMAs behind.
                while len(self.wqts) < min(n, self.kt // 2):
                    p = len(self.wqts)
                    wqt = qp.tile([P, 2, WCOLS], I8, tag="wq", name=f"{self.tag}q{p}")
                    src = wq[self.qbase + 2 * p : self.qbase + 2 * p + 2].rearrange(
                        "i p c -> p i c"
                    )
                    if self.qbase == 0 and p < 3:
                        # The kernel's first six weight tiles go as
                        # single-tile DMAs: each 128KB completion lands
                        # separately, so dequants 0-5 — the gates for the
                        # post-warmup restart and the first A-pass k-tiles
                        # — pace the PE even when the startup HBM burst is
                        # contended (pair-granular completions left 0.2-
                        # 1.3us arrival gaps at k2-k5 on warm cores).
                        nc.sync.dma_start(wqt[:, 0, :], src[:, 0, :])
                        nc.sync.dma_start(wqt[:, 1, :], src[:, 1, :])
                    else:
                        nc.sync.dma_start(wqt[:], src)
                    self.wqts.append(wqt)

            def deq_upto(self, n):
                while self.ndeq < min(n, self.kt):
                    k = self.ndeq
                    self.issue_pairs(k // 2 + 1)
                    w = wp.tile([P, 1, WCOLS], DT, tag="w", name=f"{self.tag}w{k}")
                    deqop(w[:, 0, :], self.wqts[k // 2][:, k % 2, :], self.qbase + k)
                    self.wt[k] = (w, 0)
                    self.ndeq += 1

            def mms_a(self, rhs_tiles, lead=4, plead=3, hook=None):
                """A pass (j0-5, 642ns/k-tile) with inline issue+dequant;
                dequanted tiles are RETAINED for the B pass."""
                for k in range(self.kt):
                    self.issue_pairs(k // 2 + plead)
                    self.deq_upto(k + lead)
                    if hook is not None:
                        hook(k)
                    w, kk = self.wt[k]
                    for j in JA:
                        nc.tensor.matmul(
                            self.ps[j][:],
                            mmcast(w[:, kk, j * P : (j + 1) * P]),
                            mmcast(rhs_tiles[k]),
                            start=(k == 0),
                            stop=(k == self.kt - 1),
                        )

            def mms_b(self, rhs_tiles):
                """B pass (j6-7) on the retained tiles: no dequant demand,
                so DVE builds lead for the next wave."""
                for k in range(self.kt):
                    w, kk = self.wt[k]
                    for j in JB:
                        nc.tensor.matmul(
                            self.ps[j][:],
                            mmcast(w[:, kk, j * P : (j + 1) * P]),
                            mmcast(rhs_tiles[k]),
                            start=(k == 0),
                            stop=(k == self.kt - 1),
                        )

            def epis(self, js, bias, out_dt):
                return [
                    epilogue(self.ps[j], bias, self.bias_off, j, RELU, out_dt, self.tag)
                    for j in js
                ]

        # Every wave runs as a 6-tile A pass then a 2-tile B pass over the
        # same retained weight tiles. A pass's epilogues drain on Scalar
        # during the following pass's compute, so no pass ever WARs an
        # in-flight epilogue chain — the 8-at-once wave boundary used to
        # stall the PE 2-3.5us (and trip HAM derates) while its restart
        # trailed the epilogue cadence.
        w1a = QWave(0, "l1w0", 0, kt=KT1)
        w1a.deq_upto(4)
        w1a.mms_a(xts)
        w1a.mms_b(xts)
        w1b = QWave(8, "l1w1", KT1, kt=KT1)
        w1b.deq_upto(2)
        hts = w1a.epis(JA, b1s, DT) + w1a.epis(JB, b1s, DT)
        w1b.mms_a(xts)
        w1b.mms_b(xts)
        qw1 = QWave(0, "l2w0", Q_FC2)
        qw1.deq_upto(4)
        hts += w1b.epis(JA, b1s, DT) + w1b.epis(JB, b1s, DT)
        qw1.mms_a(hts)
        qw1.mms_b(hts)

        qw2 = QWave(8, "l2w1", Q_FC2 + KT2)
        qw2.deq_upto(4)
        h2s = qw1.epis(JA, b2s, DT) + qw1.epis(JB, b2s, DT)

        # fc3 int8 prefetch interleaved into fc2 wave 2's issue stream on
        # Sync (one q3 pair per odd k): data lands during fc2 compute
        # without delaying fc2's own pairs.
        q3tiles = []

        def q3_hook(k):
            if k % 2 == 1 and len(q3tiles) < KT3 // 2:
                p = len(q3tiles)
                t3 = q3p.tile([P, 2, WCOLS], I8, tag=f"q3_{p}", name=f"q3_{p}", bufs=1)
                src = wq[Q_FC3 + 2 * p : Q_FC3 + 2 * p + 2].rearrange("i p c -> p i c")
                nc.sync.dma_start(t3[:], src)
                q3tiles.append(t3)

        qw2.mms_a(hts, hook=q3_hook)
        qw2.mms_b(hts)

        w3bf = {}

        def deq3(k):
            w = wp.tile([P, 1, WCOLS], DT, tag=f"w3_{k}", name=f"w3_{k}", bufs=1)
            deqop(w[:, 0, :], q3tiles[k // 2][:, k % 2, :], Q_FC3 + k)
            w3bf[k] = (w, 0)

        # fc3 wave A's first dequants run on DVE right after fc2 wave 2's
        # (during fc2's B pass), while the PE is still in fc2.
        for k in range(4):
            deq3(k)
        h2s += qw2.epis(JA, b2s, DT) + qw2.epis(JB, b2s, DT)

        # fc3: wave A = j0-5 k-outer (6 matmuls = 642ns per k-tile vs the
        # ~700ns dequant, absorbed by the 4-tile lead); wave B = j6-7
        # k-inner reusing the now-resident bf16 tiles. Wave A's epilogues
        # + gpsimd stores hide under wave B's 3.4us of matmuls; the tail
        # is 2 parallel epilogues + 2 parallel HW-queue stores.
        psA = [pp.tile([P, BSH], F32, tag=f"ps{j}", name=f"l3ps{j}") for j in JA]
        for k in range(KT3):
            if k + 4 < KT3:
                deq3(k + 4)
            w, kk = w3bf[k]
            for jj, j in enumerate(JA):
                nc.tensor.matmul(
                    psA[jj][:],
                    mmcast(w[:, kk, j * P : (j + 1) * P]),
                    mmcast(h2s[k]),
                    start=(k == 0),
                    stop=(k == KT3 - 1),
                )
        for jj, j in enumerate(JA):
            o = epilogue(psA[jj], b3s, 0, j, None, ODT, "l3")
            # j4/j5 store via the (idle) HW queues: the gpsimd software
            # queue's ~1.3us completion latency on the last A stores was
            # poking past wave B into the tail's completion wait.
            if j < 4:
                nc.gpsimd.dma_start(outT[j], o)
            else:
                (nc.sync if j == 4 else nc.scalar).dma_start(outT[j], o)

        # fc3's B pass runs j7's full k-loop FIRST, then j6's: j7's
        # (slower, VectorE) epilogue and its Sync store hide under j6's
        # 1.7us of matmuls, so the exec-critical tail is only j6's scalar
        # epilogue + Scalar-queue store + DMA completion.
        psB = {j: pp.tile([P, BSH], F32, tag=f"ps{j}", name=f"l3ps{j}") for j in JB}
        for j, eng, store_q in ((7, "vector", nc.sync), (6, "scalar", nc.scalar)):
            for k in range(KT3):
                w, kk = w3bf[k]
                nc.tensor.matmul(
                    psB[j][:],
                    mmcast(w[:, kk, j * P : (j + 1) * P]),
                    mmcast(h2s[k]),
                    start=(k == 0),
                    stop=(k == KT3 - 1),
                )
            o = epilogue(psB[j], b3s, 0, j, None, ODT, "l3", eng)
            store_q.dma_start(outT[j], o)


class _LightTailTileContext(tile.TileContext):
    """TileContext with a minimal end-of-kernel sequence.

    Tile's default tail (drain + full all-engine barrier + DMA/semaphore
    reset + second barrier) costs ~8-10us on HW, dominated by NRT's
    expansion of the drain-with-sem-range reset. For a single-TileContext
    kernel the correctness requirement at the end is just: all engines done
    and all output DMAs complete before the NEFF signals completion.
    """

    def _drain_and_barrier(self, tick_clock, wait_clock):
        if not hasattr(self.nc, "_tile_sem_poison_stack"):
            return super()._drain_and_barrier(tick_clock, wait_clock)
        from concourse.vector_clock import ScopedClock

        drain_inst = self.nc.sync.drain()
        wait_clock.add_sem_waits(
            drain_inst.ins, ScopedClock({None: tick_clock.global_clock})
        )
        # No trailing all-engine barrier: NEFF completion already waits
        # for every engine's stream end, and Sync's drain (with the full
        # semaphore wait list above) covers all tracked DMA completions.
        # The barrier's two $S[2] rendezvous rounds cost ~0.7us in-window.
        assert self.sems is not None
        popped = self.nc._tile_sem_poison_stack.pop()
        assert popped is self._sem_poison


def _build(dt_name):
    if dt_name in _CACHE:
        return _CACHE[dt_name]
    DT = {"bf16": mybir.dt.bfloat16, "f32r": mybir.dt.float32r, "f32": F32}[dt_name]
    MMDT = None

    patches = []
    if FAST_CONST:
        try:
            import concourse.bass as cbass

            # During Bass construction only, reroute GpSimd memsets (the
            # framework's const-AP init) to the much faster VectorE: they
            # gate the initial all-engine barrier.
            gps_cls = cbass.BassGpSimd

            def memset_shim(self, ap, constant):
                return self.bass.vector.memset(ap, constant)

            had = "memset" in vars(gps_cls)
            orig = vars(gps_cls).get("memset")
            gps_cls.memset = memset_shim
            patches.append((gps_cls, "memset", had, orig))
            # The barrier after const-AP init protects readers of the const
            # tiles; this kernel never reads them, so skip it.
            bar_orig = cbass.Bass.all_engine_barrier

            def bar_shim(self, *, sem_only=False):
                return None

            cbass.Bass.all_engine_barrier = bar_shim
            patches.append((cbass.Bass, "all_engine_barrier", True, bar_orig))
        except AttributeError:
            pass

    try:
        nc = bacc.Bacc(
            "TRN2",
            target_bir_lowering=False,
            debug=False,
            enable_asserts=False,
            num_devices=NCORES,
        )
    finally:
        for klass, attr, had, orig in patches:
            if had:
                setattr(klass, attr, orig)
            else:
                delattr(klass, attr)

    tc_cls = _LightTailTileContext if LIGHT_TAIL else tile.TileContext
    with tc_cls(nc) as tc:
        _emit(tc, DT, MMDT)
    nc.compile()
    _CACHE[dt_name] = nc
    return nc


def _np_dt(dt_name):
    return mybir.dt.np({"bf16": mybir.dt.bfloat16, "f32r": F32, "f32": F32}[dt_name])


def _host_prep(x, W1, b1, crow_indices, col_indices, values, b2, W3, b3, npdt):
    rb = crow_indices.shape[0] - 1
    nnz, bs, _ = values.shape
    cb = H // bs
    # Scatter BSR into dense W2 [H, H].
    blocks = np.zeros((rb, cb, bs, bs), np.float32)
    row_ids = (
        np.searchsorted(crow_indices, np.arange(nnz, dtype=np.int64), side="right") - 1
    )
    blocks[row_ids, col_indices] = values
    W2 = blocks.transpose(0, 2, 1, 3).reshape(H, H)

    # Pack the streamed weight sequences: for each layer, for each wave
    # (column-half), the k-tiles [P, WCOLS] in consumption order.
    def waves(wT, kdim, nw, dt):  # wT [kdim, ndim] -> [nw*kt, P, WCOLS]
        kt = kdim // P
        t = wT.reshape(kt, P, nw, WCOLS).astype(dt)
        return np.ascontiguousarray(t.transpose(2, 0, 1, 3).reshape(nw * kt, P, WCOLS))

    # ALL weights stream as int8 with per-[tile,row] max-abs scales: the
    # contended HBM weight bytes drop 4x; dequant to bf16 is on-device.
    wq_f = np.concatenate(
        [
            waves(np.ascontiguousarray(W1.T), IN, NW1, np.float32),
            waves(np.ascontiguousarray(W2.T), H, NW2, np.float32),
            waves(np.ascontiguousarray(W3.T), H, NW3, np.float32),
        ]
    )  # [64, P, WCOLS] fp32
    sc = np.abs(wq_f).max(axis=2) / 127.0  # [64, P]
    sc = np.maximum(sc, 1e-30)
    wq8 = np.clip(np.rint(wq_f / sc[:, :, None]), -127, 127).astype(np.int8)
    bc = np.ascontiguousarray(
        np.concatenate(
            [
                b1.reshape(H // P, P).T.astype(np.float32),
                b2.reshape(H // P, P).T.astype(np.float32),
                b3.reshape(OUT // P, P).T.astype(np.float32),
                sc.T.astype(np.float32),
            ],
            axis=1,
        )
    )
    # x -> per-core transposed shards, [P, kt, BSH], natural k order.
    xT_all = np.ascontiguousarray(x.T.astype(npdt))  # [IN, B]
    shards = [
        np.ascontiguousarray(
            xT_all[:, c * BSH : (c + 1) * BSH].reshape(KT1, P, BSH).transpose(1, 0, 2)
        )
        for c in range(NCORES)
    ]
    shared = dict(wq=wq8, bc=bc)
    return [dict(shared, xT=shards[c]) for c in range(NCORES)]


def kernel(x, W1, b1, crow_indices, col_indices, values, b2, W3, b3, _dt="bf16"):
    nc = _build(_dt)
    in_maps = _host_prep(
        np.asarray(x, np.float32),
        np.asarray(W1, np.float32),
        np.asarray(b1, np.float32),
        np.asarray(crow_indices),
        np.asarray(col_indices),
        np.asarray(values, np.float32),
        np.asarray(b2, np.float32),
        np.asarray(W3, np.float32),
        np.asarray(b3, np.float32),
        _np_dt(_dt),
    )
    res = bass_utils.run_bass_kernel_spmd(nc, in_maps, core_ids=list(range(NCORES)))
    out = np.concatenate(
        [res.results[c]["outT"].reshape(OUT, BSH).T for c in range(NCORES)], axis=0
    )
    return np.ascontiguousarray(out.astype(np.float32))

